# revision 113
# baseline (speedup 1.0000x reference)
"""Bamba attention decoder layer on 8 Trainium2 NeuronCores.

Sharding: tensor-parallel attention (4 q heads + 1 kv head per core),
AllToAll of attention context (delivers each core its token slice at a static
address), token-sliced o_proj + fused add, chunked AllGather of the
*unnormalized* residual (the 1/rms factor commutes through the gate/up
matmuls and is applied on the consumer side), I-sharded SwiGLU MLP
(1792 cols/core) fused with the down projection per token half (h stays in
SBUF), ReduceScatter of down-proj partials.

Matmul operands are bf16 (full PE rate, half the DMA/SBUF footprint of
fp32r); the residual stream, rmsnorm statistics and kernel outputs stay
fp32. ln weights are folded into the weight matrices on the host; the
1/rms factor of each rmsnorm is applied after the following matmul
(per-token column scaling commutes through the contraction).
"""

import numpy as np
import ml_dtypes

import concourse.bacc as bacc
import concourse.mybir as mybir
import concourse.tile as tile
from concourse.bass_utils import run_bass_kernel_spmd
from concourse.masks import make_identity

NC = 8
S = 2048
H = 4096
HD = 128
NQ = 32
NKV = 8
I = 14336
QH = NQ // NC        # q heads per core = 4
IPC = I // NC        # intermediate cols per core = 1792
TPC = S // NC        # tokens per core = 256
EPS = 1e-5
THETA = 10000.0
SCALE = HD ** -0.5

F32 = mybir.dt.float32
F32R = mybir.dt.float32r
BF16 = mybir.dt.bfloat16

KH = H // 128        # 32 k-tiles over H
NB = S // 512        # 4 token blocks of 512
MB_GU = IPC // 128   # 14 m tiles for gate (and for up)
KI = IPC // 128      # 14 k tiles over I per core
SH = S // 2          # tokens per half = 1024
# AllGather chunking of the residual stream: smaller tail chunks so the
# last chunk's transfer chain at the o_proj->MLP boundary is short
AG_CHUNKS = [(0, 8), (8, 8), (16, 4), (20, 4), (24, 4), (28, 4)]

AF = mybir.ActivationFunctionType


def _attn_block(nc, g, p2s, p2p, hh, qb, with_collectives):
    """Causal GQA attention for head hh, q-block qb (512 q tokens)."""
    nkt = 4 * qb + 4
    att_ps = p2p.tile([128, 512], F32, name="att_ps", tag="att_ps", bufs=1)
    sums_ps = p2p.tile([1, 512], F32, name="sums_ps", tag="sums_ps", bufs=1)
    # waves of 4: issue 4 score matmuls back-to-back, then their att/sums
    # accumulations — by the time the PE (in-order) reaches an att matmul,
    # its exp+mask chain has drained, so it doesn't bubble per tile
    for w0 in range(0, nkt, 16):
        wave = range(w0, min(w0 + 16, nkt))
        es = []
        for kt in wave:
            j = kt - 4 * qb
            # on diagonal tiles only q-columns >= 128*j attend at all;
            # skip the fully-masked column range entirely
            q0 = 128 * j if j > 0 else 0
            lsl = slice(q0, 512)
            s_ps = p2p.tile([128, 512], F32, name="s_ps", tag="s_ps", bufs=4)
            nc.tensor.matmul(
                s_ps[:, lsl], g["kT_sb"][:, kt * 128:(kt + 1) * 128],
                g["qT_sb"][:, hh, qb * 512 + q0:(qb + 1) * 512],
                start=True, stop=True,
            )
            e = p2s.tile([128, 512], BF16, name="e", tag="e", bufs=8)
            nc.scalar.activation(e[:, lsl], s_ps[:, lsl], AF.Exp, scale=SCALE)
            if j >= 0:
                nc.vector.tensor_mul(e[:, lsl], e[:, lsl], g["mask_sb"][:, j, lsl])
            es.append((kt, lsl, e))
        for kt, lsl, e in es:
            nc.tensor.matmul(att_ps[:, lsl], g["v_tok"][:, kt, :], e[:, lsl],
                             start=(kt == 0), stop=(kt == nkt - 1),
                             skip_group_check=True)
            nc.tensor.matmul(sums_ps[:, lsl], g["ones_b"][:], e[:, lsl],
                             start=(kt == 0), stop=(kt == nkt - 1),
                             skip_group_check=True)
    recip = p2s.tile([1, 512], F32, name="recip", tag="recip")
    nc.vector.reciprocal(recip[:], sums_ps[:])
    rb2 = p2s.tile([128, 512], F32, name="rb2", tag="rb2", bufs=2)
    nc.gpsimd.partition_broadcast(rb2[:], recip[:])
    anorm = p2s.tile([128, 512], BF16, name="anorm", tag="anorm", bufs=2)
    nc.vector.tensor_mul(anorm[:], att_ps[:], rb2[:])
    # with collectives: stage into a2a_in and AllToAll per head.
    # without (cost-model path): the "collective" degenerates to
    # writing this core's slice straight to a2a_out.
    dst = g[f"a2a_in{hh}"] if with_collectives else g[f"a2a_out{hh}"]
    for half in range(2):
        dst_core = qb * 2 + half
        nc.sync.dma_start(
            dst[dst_core * 128:(dst_core + 1) * 128, :],
            anorm[:, half * 256:(half + 1) * 256],
        )


def _phase12_qkv_attn(nc, tc, g, p2s, with_collectives, rg):
    p1p = g["psum"]
    """QKV matmul + rope, fused with attention: the attention for token
    block nb runs right after block nb's rope, filling the PE while the
    exp/softmax pipeline of earlier blocks drains.  The rmsnorm1 factor is
    precomputed on the host (it only depends on the kernel input) and folded
    into cosT/sinT; v is scaled by the hosted rstd1 row directly."""
    with (
        tc.tile_pool(name="p1sbuf", bufs=2) as p1s,
        tc.tile_pool(name="p1w", bufs=1) as p1w,
    ):
        # first-needed-first DMA order: block-0 activations + first weight
        # chunk go ahead of everything else so the PE starts within ~5us.
        hb_pre = []
        for kc in range(4):
            hb_pre.append(p1s.tile([128, 8, 512], BF16, name="hb", tag="hb", bufs=6))
        wq_m = []
        for m in range(QH + 2):
            wq_m.append(p1w.tile([128, KH, 128], BF16, name=f"wq{m}"))
        # m-major weight chunks interleaved with block-0 activations: m0's
        # first weights + the first activation chunk arrive within ~2us
        nc.sync.dma_start(hb_pre[0][:, 0:4, :], g["hTb"][:, 0:4, 0:512])
        nc.sync.dma_start(wq_m[0][:], g["wqkv"][:, 0, :, :])
        nc.sync.dma_start(hb_pre[0][:, 4:8, :], g["hTb"][:, 4:8, 0:512])
        nc.sync.dma_start(wq_m[1][:], g["wqkv"][:, 1, :, :])
        nc.sync.dma_start(hb_pre[1][:], g["hTb"][:, 8:16, 0:512])
        nc.sync.dma_start(hb_pre[2][:], g["hTb"][:, 16:24, 0:512])
        nc.sync.dma_start(hb_pre[3][:], g["hTb"][:, 24:32, 0:512])
        for m in range(2, QH + 2):
            nc.sync.dma_start(wq_m[m][:], g["wqkv"][:, m, :, :])
        cos_sb = p1w.tile([128, S], BF16, name="cos_sb")
        nc.sync.dma_start(cos_sb[:], g["cosT"][:, :])
        sin_sb = p1w.tile([128, S], BF16, name="sin_sb")
        nc.sync.dma_start(sin_sb[:], g["sinT"][:, :])
        rstd1 = g["rstd1_sb"]
        nc.sync.dma_start(rstd1[:], g["rstd1"][:, :])
        nc.sync.dma_start(g["mask_sb"][:], g["masks"][:, :, :])

        for nb in range(NB):
            ncols = slice(nb * 512, (nb + 1) * 512)
            if nb == 0:
                hbs = hb_pre
            else:
                hbs = []
                for kc in range(4):
                    hb = p1s.tile([128, 8, 512], BF16, name="hb", tag="hb", bufs=6)
                    nc.sync.dma_start(hb[:], g["hTb"][:, kc * 8:(kc + 1) * 8, ncols])
                    hbs.append(hb)
            rb = p1s.tile([128, 512], F32, name="rb", tag="rb", bufs=2)
            nc.gpsimd.partition_broadcast(rb[:], rstd1[:, ncols])

            def finish_m(m, mm):
                if m < QH + 1:
                    qkc = p1s.tile([128, 512], F32, name="qkc", tag="qkc", bufs=2)
                    nc.scalar.copy(qkc[:], mm[:])
                    if m < QH:
                        d0 = g["qT_sb"][0:64, m, ncols]
                        d1 = g["qT_sb"][64:128, m, ncols]
                    else:
                        d0 = g["kT_sb"][0:64, ncols]
                        d1 = g["kT_sb"][64:128, ncols]
                    t0 = p1s.tile([64, 512], F32, name="t0", tag="t0", bufs=1)
                    nc.vector.tensor_mul(t0[:], qkc[0:64, :], cos_sb[0:64, ncols])
                    t1 = p1s.tile([64, 512], F32, name="t1", tag="t1", bufs=1)
                    nc.vector.tensor_mul(t1[:], qkc[64:128, :], sin_sb[64:128, ncols])
                    nc.vector.tensor_sub(d0, t0[:], t1[:])
                    t2 = p1s.tile([64, 512], F32, name="t2", tag="t0", bufs=1)
                    nc.vector.tensor_mul(t2[:], qkc[64:128, :], cos_sb[64:128, ncols])
                    t3 = p1s.tile([64, 512], F32, name="t3", tag="t1", bufs=1)
                    nc.vector.tensor_mul(t3[:], qkc[0:64, :], sin_sb[0:64, ncols])
                    nc.vector.tensor_add(d1, t2[:], t3[:])
                else:
                    vtmp = p1s.tile([128, 512], BF16, name="vtmp", tag="vtmp", bufs=1)
                    nc.vector.tensor_mul(vtmp[:], mm[:], rb[:])
                    for j in range(4):
                        # shares the attention-score psum tag (bank budget)
                        tp = p1p.tile([128, 128], BF16, name="tp", tag="s_ps",
                                      bufs=4)
                        nc.tensor.transpose(tp[:], vtmp[:, j * 128:(j + 1) * 128],
                                            g["ident"][:])
                        nc.vector.tensor_copy(g["v_tok"][:, nb * 4 + j, :], tp[:])

            # m-outer: one accumulating psum at a time (block-resident hb);
            # the previous block's attention interleaves between this
            # block's qkv chains, so each ACT-paced attention unit is
            # followed by a dense matmul chain and the in-order PE doesn't
            # idle on the exp pipeline
            for m in range(QH + 2):
                mm = p1p.tile([128, 512], F32, name="mm", tag="mm", bufs=2)
                for kc in range(4):
                    for kk in range(8):
                        k = kc * 8 + kk
                        nc.tensor.matmul(
                            mm[:], wq_m[m][:, k, :], hbs[kc][:, kk, :],
                            start=(k == 0), stop=(k == KH - 1),
                        )
                finish_m(m, mm)
                if nb > 0 and m < QH:
                    _attn_block(nc, g, p2s, p1p, m, nb - 1, with_collectives)
            if nb == 1:
                # the attention stretch needs almost no DMA: prefetch the
                # first o_proj weight tiles into their reserved pool now
                for i in range(len(g["wo_pre"])):
                    nc.sync.dma_start(g["wo_pre"][i][:], g["wo"][:, i, :, :])
        # the last block's attention runs after its rope; ship each head's
        # context as it completes
        for hh in range(QH):
            _attn_block(nc, g, p2s, p1p, hh, NB - 1, with_collectives)
            if with_collectives:
                nc.gpsimd.collective_compute(
                    "AllToAll", mybir.AluOpType.bypass, replica_groups=rg,
                    ins=[g[f"a2a_in{hh}"].opt()], outs=[g[f"a2a_out{hh}"].opt()],
                )


def _phase3_oproj(nc, tc, g, with_collectives, rg):
    """Token-sliced o_proj + residual add.  Ships the *unnormalized* res2 in
    4 AllGather chunks as it is produced, plus one tiny AllGather of the
    sum-of-squares row; the 1/rms scaling happens on the consumer side.
    Also begins loading the first token half of the gathered activations
    (g["xn"]) on the Pool DMA queue as chunks land."""
    with (
        tc.tile_pool(name="p3sbuf", bufs=2) as p3s,
        tc.tile_pool(name="p3big", bufs=1) as p3b,
    ):
        p3p = g["psum"]
        # asl k' order: k' = hh*8 + r  <->  global q head g = r*4 + hh
        asl = p3b.tile([128, KH, TPC], BF16, name="asl")   # 2.1 MB
        for hh in range(QH):
            for r in range(8):
                nc.sync.dma_start(asl[:, hh * 8 + r, :],
                                  g[f"a2a_out{hh}"][r * 128:(r + 1) * 128, :])
        xn = g["xn"]
        st2_ps = p3p.tile([1, TPC], F32, name="st2_ps", tag="sums_ps", bufs=1)
        for cch, (off, sz) in enumerate(AG_CHUNKS):
            hsl = p3s.tile([128, sz, TPC], BF16, name="hsl", tag="hsl", bufs=2)
            nc.sync.dma_start(hsl[:], g["hT_slice"][:, off:off + sz, :])
            res2 = p3s.tile([128, sz, TPC], F32, name="res2", tag="res2", bufs=1)
            x2b = p3b.tile([128, sz, TPC], BF16, name=f"x2b{cch}")
            g[f"x2b{cch}"] = x2b
            for mi in range(sz):
                m = off + mi
                if m < len(g["wo_pre"]):
                    wob = g["wo_pre"][m]   # prefetched during attention
                else:
                    wob = p3s.tile([128, KH, 128], BF16, name="wob", tag="wob", bufs=4)
                    nc.sync.dma_start(wob[:], g["wo"][:, m, :, :])
                o_ps = p3p.tile([128, TPC], F32, name="o_ps", tag="s_ps", bufs=4)
                for k in range(KH):
                    nc.tensor.matmul(o_ps[:], wob[:, k, :], asl[:, k, :],
                                     start=(k == 0), stop=(k == KH - 1))
                nc.vector.tensor_add(res2[:, mi, :], o_ps[:], hsl[:, mi, :])
                sq2 = p3s.tile([128, TPC], F32R, name="sq2", tag="sq2", bufs=2)
                nc.scalar.activation(sq2[:], res2[:, mi, :], AF.Square)
                nc.tensor.matmul(st2_ps[:], g["ones"][:], sq2[:],
                                 start=(m == 0), stop=(m == KH - 1))
                nc.vector.tensor_copy(x2b[:, mi, :], res2[:, mi, :])
                if m == 0:
                    # prefetch the first gate/up weight pair off the Pool
                    # queue so it's resident when the MLP starts
                    nc.gpsimd.dma_start(g["gb0"][:], g["wgu"][:, 0, :, :])
                    nc.gpsimd.dma_start(g["ub0"][:], g["wgu"][:, MB_GU, :, :])
            # AllGather chain first (it feeds the MLP's critical path),
            # res_out write after.  Fallback writes the core's own slice
            # straight to ag2_out, skipping the staging copy.
            if with_collectives:
                nc.sync.dma_start(g[f"ag2_in{cch}"][:, :, :], x2b[:, :, :])
                nc.gpsimd.collective_compute(
                    "AllGather", mybir.AluOpType.bypass, replica_groups=rg,
                    ins=[g[f"ag2_in{cch}"].opt()], outs=[g[f"ag2_out{cch}"].opt()],
                )
            else:
                nc.sync.dma_start(g[f"ag2_out{cch}"][0:128, :, :], x2b[:, :, :])
            # token half 0 of the later chunks -> SBUF, off the Pool DMA
            # queue so it rides during o_proj without blocking SP.  Chunks
            # 0-1 are deferred to the MLP start (the contraction consumes
            # them last) to relieve o_proj's saturated DMA window.
            for cp in range(4):
                nc.gpsimd.dma_start(
                    xn[:, off:off + sz, cp * 256:(cp + 1) * 256],
                    g[f"ag2_out{cch}"][cp * 128:(cp + 1) * 128, :, :],
                )
        # ship the raw sum-of-squares row; every core derives rstd locally
        sums_sb = p3s.tile([1, TPC], F32, name="sums_sb", tag="sums_sb")
        nc.vector.tensor_copy(sums_sb[:], st2_ps[:])
        if with_collectives:
            nc.sync.dma_start(g["sums_in"][:, :], sums_sb[:])
            nc.gpsimd.collective_compute(
                "AllGather", mybir.AluOpType.bypass, replica_groups=rg,
                ins=[g["sums_in"].opt()], outs=[g["sums_out"].opt()],
            )
        else:
            nc.sync.dma_start(g["sums_out"][0:1, :], sums_sb[:])


def _phase45_mlp(nc, tc, g, with_collectives, rg):
    """Fused SwiGLU + down projection, one token half at a time.
    h never leaves SBUF; the 1/rms factor of rmsnorm2 is applied to the
    gate/up psums (it commutes through the contraction).  ReduceScatter
    chunks (by output-row group) fire as the second half completes them."""
    with (
        tc.tile_pool(name="p45w", bufs=1) as pw,
        tc.tile_pool(name="p45big", bufs=1) as pb45,
        tc.tile_pool(name="p45sbuf", bufs=2) as ps,
    ):
        pp = g["psum"]
        g["h"] = pb45.tile([128, KI, SH], BF16, name="h")    # 3.7 MB
        # rstd for all tokens from the gathered sum-of-squares rows
        sums8 = pw.tile([8, TPC], F32, name="sums8")
        nc.sync.dma_start(sums8[:], g["sums_out"][:, :])
        std8 = pw.tile([8, TPC], F32, name="std8")
        nc.scalar.activation(std8[:], sums8[:], AF.Sqrt,
                             bias=g["epsb8"][:], scale=1.0 / H)
        rstd8 = pw.tile([8, TPC], F32, name="rstd8")
        nc.vector.reciprocal(rstd8[:], std8[:])
        nc.sync.dma_start(g["rstd_dram"][0, :, :], rstd8[:])
        rstd_row = g["rstd1_sb"]   # rmsnorm1's row tile, dead after phase 1
        nc.sync.dma_start(rstd_row[:], g["rstd_dram"][0:1, :, :])

        # residual output, deferred out of o_proj's DMA-saturated window
        for cch, (off, sz) in enumerate(AG_CHUNKS):
            nc.sync.dma_start(g["res_out"][:, off:off + sz, :], g[f"x2b{cch}"][:])

        xn = g["xn"]
        h = g["h"]
        for T in range(2):
            tsl = slice(T * SH, (T + 1) * SH)
            rbh = ps.tile([128, SH], F32, name="rbh", tag="rbh", bufs=1)
            nc.gpsimd.partition_broadcast(rbh[:], rstd_row[:, tsl])
            if T == 1:
                # second token half of the gathered activations; WAR on xn
                # delays these harmlessly on the Pool queue until the first
                # half's matmuls finish
                for cch, (off, sz) in enumerate(AG_CHUNKS):
                    for cp in range(4):
                        nc.gpsimd.dma_start(
                            xn[:, off:off + sz, cp * 256:(cp + 1) * 256],
                            g[f"ag2_out{cch}"][(4 + cp) * 128:(5 + cp) * 128, :, :],
                        )
            # gate/up.  The first two weight tiles of half 0 go out on the
            # ACT DMA queue: the SP queue is still draining o_proj-tail
            # writes when the PE becomes ready for them.
            for m in range(MB_GU):
                if T == 0 and m == 0:
                    gb, ub = g["gb0"], g["ub0"]   # preloaded during o_proj
                else:
                    gb = ps.tile([128, KH, 128], BF16, name="gb", tag="wgu", bufs=3)
                    nc.sync.dma_start(gb[:], g["wgu"][:, m, :, :])
                    ub = ps.tile([128, KH, 128], BF16, name="ub", tag="wgu", bufs=3)
                    nc.sync.dma_start(ub[:], g["wgu"][:, MB_GU + m, :, :])
                for tb in range(2):
                    tcols = slice(tb * 512, (tb + 1) * 512)
                    g_ps = pp.tile([128, 512], F32, name="g_ps", tag="s_ps", bufs=4)
                    for k in range(KH):
                        nc.tensor.matmul(g_ps[:], gb[:, k, :], xn[:, k, tcols],
                                         start=(k == 0), stop=(k == KH - 1))
                    u_ps = pp.tile([128, 512], F32, name="u_ps", tag="s_ps", bufs=4)
                    for k in range(KH):
                        nc.tensor.matmul(u_ps[:], ub[:, k, :], xn[:, k, tcols],
                                         start=(k == 0), stop=(k == KH - 1))
                    gsc = ps.tile([128, 512], F32R, name="gsc", tag="gsc", bufs=1)
                    nc.vector.tensor_mul(gsc[:], g_ps[:], rbh[:, tcols])
                    usc = ps.tile([128, 512], F32R, name="usc", tag="usc", bufs=1)
                    nc.vector.tensor_mul(usc[:], u_ps[:], rbh[:, tcols])
                    sg = ps.tile([128, 512], F32R, name="sg", tag="sg", bufs=1)
                    nc.scalar.activation(sg[:], gsc[:], AF.Silu)
                    nc.vector.tensor_mul(h[:, m, tcols], sg[:], usc[:])
            # down
            for r in range(8):
                for mi in range(KH // 8):
                    m = r * (KH // 8) + mi
                    db = ps.tile([128, KI, 128], BF16, name="db", tag="db", bufs=3)
                    nc.sync.dma_start(db[:], g["wdn"][:, m, :, :])
                    ot = ps.tile([128, SH], BF16, name="ot", tag="ot", bufs=1)
                    for tb in range(2):
                        tcols = slice(tb * 512, (tb + 1) * 512)
                        d_ps = pp.tile([128, 512], F32, name="d_ps", tag="mm", bufs=2)
                        for k in range(KI):
                            nc.tensor.matmul(d_ps[:], db[:, k, :], h[:, k, tcols],
                                             start=(k == 0), stop=(k == KI - 1))
                        nc.vector.tensor_copy(ot[:, tcols], d_ps[:])
                    nc.sync.dma_start(g[f"rs_in{r}"][mi * 128:(mi + 1) * 128, tsl], ot[:])
                if T == 1:
                    if with_collectives:
                        nc.gpsimd.collective_compute(
                            "ReduceScatter", mybir.AluOpType.add, replica_groups=rg,
                            ins=[g[f"rs_in{r}"].opt()], outs=[g[f"rs_out{r}"].opt()],
                        )
                    else:
                        nc.sync.dma_start(g[f"rs_out{r}"][:, :],
                                          g[f"rs_in{r}"][0:H // NC // 8, :])
                    # upconvert the bf16 shard to the fp32 output
                    ob = ps.tile([64, S], BF16, name="ob", tag="ob", bufs=1)
                    nc.sync.dma_start(ob[:], g[f"rs_out{r}"][:, :])
                    for hf in range(2):
                        hsl2 = slice(hf * SH, (hf + 1) * SH)
                        of = ps.tile([64, SH], F32, name="of", tag="of", bufs=1)
                        nc.vector.tensor_copy(of[:], ob[:, hsl2])
                        nc.sync.dma_start(g["out_down"][r * 64:(r + 1) * 64, hsl2], of[:])


def build_program(with_collectives=True, stop_after=99):
    nc = bacc.Bacc("TRN2", target_bir_lowering=False, debug=False, num_devices=NC)

    g = {}
    g["hTb"] = nc.dram_tensor("hTb", [128, KH, S], BF16, kind="ExternalInput")
    g["hT_slice"] = nc.dram_tensor("hT_slice", [128, KH, TPC], BF16, kind="ExternalInput")
    g["wqkv"] = nc.dram_tensor("wqkv", [128, QH + 2, KH, 128], BF16, kind="ExternalInput")
    g["wo"] = nc.dram_tensor("wo", [128, KH, KH, 128], BF16, kind="ExternalInput")
    g["wgu"] = nc.dram_tensor("wgu", [128, 2 * MB_GU, KH, 128], BF16, kind="ExternalInput")
    g["wdn"] = nc.dram_tensor("wdn", [128, KH, KI, 128], BF16, kind="ExternalInput")
    g["cosT"] = nc.dram_tensor("cosT", [128, S], BF16, kind="ExternalInput")
    g["sinT"] = nc.dram_tensor("sinT", [128, S], BF16, kind="ExternalInput")
    g["rstd1"] = nc.dram_tensor("rstd1", [1, S], F32, kind="ExternalInput")
    g["masks"] = nc.dram_tensor("masks", [128, 4, 512], BF16, kind="ExternalInput")

    g["res_out"] = nc.dram_tensor("res_out", [128, KH, TPC], BF16, kind="ExternalOutput")
    g["out_down"] = nc.dram_tensor("out_down", [H // NC, S], F32, kind="ExternalOutput")

    rg = [list(range(NC))]

    with tile.TileContext(nc) as tc:
        with (
            tc.tile_pool(name="consts", bufs=1) as consts,
            tc.tile_pool(name="dram", bufs=1, space="DRAM") as dram,
        ):
            shr = {"addr_space": "Shared"} if with_collectives else {}
            for hh in range(QH):
                g[f"a2a_in{hh}"] = dram.tile([NC * 128, TPC], BF16, name=f"a2a_in{hh}")
                g[f"a2a_out{hh}"] = dram.tile([NC * 128, TPC], BF16, name=f"a2a_out{hh}")
            for cch, (off, sz) in enumerate(AG_CHUNKS):
                g[f"ag2_in{cch}"] = dram.tile([128, sz, TPC], BF16, name=f"ag2_in{cch}")
                g[f"ag2_out{cch}"] = dram.tile([NC * 128, sz, TPC], BF16,
                                               name=f"ag2_out{cch}", **shr)
            g["sums_in"] = dram.tile([1, TPC], F32, name="sums_in")
            g["sums_out"] = dram.tile([NC, TPC], F32, name="sums_out", addr_space="Shared")
            g["rstd_dram"] = dram.tile([1, NC, TPC], F32, name="rstd_dram")
            for r in range(8):
                g[f"rs_in{r}"] = dram.tile([H // 8, S], BF16, name=f"rs_in{r}")
                g[f"rs_out{r}"] = dram.tile([H // NC // 8, S], BF16, name=f"rs_out{r}")

            ones32 = consts.tile([128, 1], F32, name="ones32")
            nc.gpsimd.memset(ones32[:], 1.0)
            g["ones"] = consts.tile([128, 1], F32R, name="ones")
            nc.vector.tensor_copy(g["ones"][:], ones32[:])
            g["ones_b"] = consts.tile([128, 1], BF16, name="ones_b")
            nc.vector.tensor_copy(g["ones_b"][:], ones32[:])
            ident32 = consts.tile([128, 128], F32, name="ident32")
            make_identity(nc, ident32[:])
            g["ident"] = consts.tile([128, 128], BF16, name="ident")
            nc.vector.tensor_copy(g["ident"][:], ident32[:])
            g["epsb"] = consts.tile([1, 1], F32, name="epsb")
            nc.gpsimd.memset(g["epsb"][:], EPS)
            g["epsb8"] = consts.tile([8, 1], F32, name="epsb8")
            nc.gpsimd.memset(g["epsb8"][:], EPS)
            g["rstd1_sb"] = consts.tile([1, S], F32, name="rstd1_sb")

            # reserved early so o_proj's first weight tiles can prefetch
            # during the (DMA-idle) attention stretch
            psum_cm = tc.tile_pool(name="psum", bufs=1, space="PSUM")
            g["psum"] = psum_cm.__enter__()
            wop_cm = tc.tile_pool(name="wopre", bufs=1)
            wop = wop_cm.__enter__()
            g["wo_pre"] = [wop.tile([128, KH, 128], BF16, name=f"wopre{i}")
                           for i in range(4)]

            with tc.tile_pool(name="attn", bufs=1) as attn:
                g["mask_sb"] = attn.tile([128, 4, 512], BF16, name="mask_sb")
                g["qT_sb"] = attn.tile([128, QH, S], BF16, name="qT_sb")          # 2 MB
                g["kT_sb"] = attn.tile([128, S], BF16, name="kT_sb")              # 0.5 MB
                g["v_tok"] = attn.tile([128, S // 128, 128], BF16, name="v_tok")  # 0.5 MB

                with tc.tile_pool(name="p2sbuf", bufs=2) as p2s:
                    _phase12_qkv_attn(nc, tc, g, p2s, with_collectives, rg)

            if stop_after >= 3:
                with tc.tile_pool(name="mlpbig", bufs=1) as pb:
                    g["xn"] = pb.tile([128, KH, SH], BF16, name="xn")   # 8.4 MB
                    g["gb0"] = pb.tile([128, KH, 128], BF16, name="gb0")
                    g["ub0"] = pb.tile([128, KH, 128], BF16, name="ub0")
                    _phase3_oproj(nc, tc, g, with_collectives, rg)
                    if stop_after >= 4:
                        _phase45_mlp(nc, tc, g, with_collectives, rg)
            wop_cm.__exit__(None, None, None)
            psum_cm.__exit__(None, None, None)

    nc.finalize()
    return nc


_cached_nc = None


def _get_nc():
    global _cached_nc
    if _cached_nc is None:
        _cached_nc = build_program(with_collectives=True)
    return _cached_nc


def _host_prep(positions, hidden_states, w_qkv, w_o, w_gate_up, w_down, ln1_w, ln2_w):
    f32 = np.float32
    bf16 = ml_dtypes.bfloat16
    hidden = np.asarray(hidden_states, dtype=f32)[0]          # [S, H]
    hT = np.ascontiguousarray(hidden.T)                        # [H, S]
    hTb_np = np.ascontiguousarray(
        hT.reshape(KH, 128, S).transpose(1, 0, 2)).astype(bf16)  # [128, KH, S]
    pos = np.asarray(positions).astype(f32)[0]                 # [S]

    half = HD // 2
    inv_freq = (1.0 / (f32(THETA) ** (np.arange(0, half, dtype=f32) / f32(half)))).astype(f32)
    ang = pos[:, None] * inv_freq[None, :]                     # [S, 64] fp32
    # rmsnorm1 only depends on the input: precompute 1/rms per token and
    # fold it into the rope tables (it commutes through the QKV matmul)
    rstd1_np = (1.0 / np.sqrt((hidden.astype(np.float64) ** 2).mean(axis=1) + EPS)
                ).astype(f32)                                  # [S]
    cos_half = (np.cos(ang).astype(f32) * rstd1_np[:, None]).T  # [64, S]
    sin_half = (np.sin(ang).astype(f32) * rstd1_np[:, None]).T
    cosT_np = np.ascontiguousarray(np.concatenate([cos_half, cos_half], axis=0)).astype(bf16)  # [128, S]
    sinT_np = np.ascontiguousarray(np.concatenate([sin_half, sin_half], axis=0)).astype(bf16)
    rstd1_row = np.ascontiguousarray(rstd1_np.reshape(1, S))

    w_qkv_f = np.asarray(w_qkv, dtype=f32) * np.asarray(ln1_w, dtype=f32)[:, None]
    w_gu_f = np.asarray(w_gate_up, dtype=f32) * np.asarray(ln2_w, dtype=f32)[:, None]
    # o_proj contraction order k' = hh*8 + r  <->  global head g = r*4 + hh
    w_o_r = np.asarray(w_o, dtype=f32).reshape(KH, 128, KH, 128)
    g_of_kp = [(kp % 8) * 4 + kp // 8 for kp in range(KH)]
    w_o_f = np.ascontiguousarray(
        w_o_r[g_of_kp].transpose(1, 2, 0, 3)).astype(bf16)     # [128, m, k', 128]
    w_dn_f = np.asarray(w_down, dtype=f32)

    kk = np.arange(128)[:, None, None]
    jj = np.arange(4)[None, :, None]
    qq = np.arange(512)[None, None, :]
    masks_np = np.ascontiguousarray((qq >= kk + 128 * jj).astype(bf16))  # [128, 4, 512]

    in_maps = []
    for c in range(NC):
        q_cols = w_qkv_f[:, c * QH * HD:(c + 1) * QH * HD]
        k_col = w_qkv_f[:, NQ * HD + c * HD: NQ * HD + (c + 1) * HD]
        v_col = w_qkv_f[:, (NQ + NKV) * HD + c * HD: (NQ + NKV) * HD + (c + 1) * HD]
        wqkv_c = np.concatenate([q_cols, k_col, v_col], axis=1)
        wqkv_c = np.ascontiguousarray(
            wqkv_c.reshape(KH, 128, QH + 2, 128).transpose(1, 2, 0, 3)).astype(bf16)
        wgu_c = np.concatenate(
            [w_gu_f[:, c * IPC:(c + 1) * IPC],
             w_gu_f[:, I + c * IPC: I + (c + 1) * IPC]], axis=1)
        wgu_c = np.ascontiguousarray(
            wgu_c.reshape(KH, 128, 2 * MB_GU, 128).transpose(1, 2, 0, 3)).astype(bf16)
        wdn_c = np.ascontiguousarray(
            w_dn_f[c * IPC:(c + 1) * IPC, :].reshape(KI, 128, KH, 128).transpose(1, 2, 0, 3)
        ).astype(bf16)
        hT_slice_c = np.ascontiguousarray(
            hT[:, c * TPC:(c + 1) * TPC].reshape(KH, 128, TPC).transpose(1, 0, 2)
        ).astype(bf16)
        in_maps.append({
            "hTb": hTb_np,
            "hT_slice": hT_slice_c,
            "wqkv": wqkv_c,
            "wo": w_o_f,
            "wgu": wgu_c,
            "wdn": wdn_c,
            "cosT": cosT_np,
            "sinT": sinT_np,
            "rstd1": rstd1_row,
            "masks": masks_np,
        })
    return in_maps


def kernel(**inputs):
    in_maps = _host_prep(**inputs)
    nc = _get_nc()
    res = run_bass_kernel_spmd(nc, in_maps, core_ids=list(range(NC)))
    results = res.results

    outT = np.empty((H, S), np.float32)
    for c in range(NC):
        od = results[c]["out_down"]           # [512, S]: chunk r rows -> global 512r+64c
        for r in range(8):
            outT[512 * r + 64 * c: 512 * r + 64 * (c + 1)] = od[64 * r:64 * (r + 1)]
    resT = np.concatenate(
        [np.asarray(results[c]["res_out"]).astype(np.float32)
         .transpose(1, 0, 2).reshape(H, TPC)
         for c in range(NC)], axis=1)          # [H, S]
    out = np.ascontiguousarray(outT.T).reshape(1, S, H).astype(np.float32)
    residual = np.ascontiguousarray(resT.T).reshape(1, S, H).astype(np.float32)
    return out, residual
